# revision 5
# baseline (speedup 1.0000x reference)
"""Trainium2 Bass kernel for nn_ContrastLoss (smooth-histogram contrast loss).

Algorithm
---------
reference computes, per image:  hist[b] = sum_p w(x_p,b) / (S_p + 1e-8),
w = exp(-0.5*((x - c_b)/sigma)^2), c_b = b/255, sigma = 0.01, S_p = sum_b w,
followed by MSEs between the three histograms.

hist is a fixed linear map of the count histogram of u = round(x * 255)
in [0, 255] (256 levels = the bin centers themselves; quantization error on
the loss is ~5e-4 rel, far inside tolerance):
    hist[b] = sum_u cnt[u] * Phi[u, b]
The device only needs cnt[256] per image — a pure counting problem.

Device kernel (SPMD over 8 cores, data-parallel over pixels):
  - per core/image, 32768 pixels in SBUF [128, 256]; u = round(255 x) via the
    2^23 magic-add on ACT; split u = 16*hi + lo (hi via a second magic-add,
    lo via one scalar_tensor_tensor on Pool).
  - counting via PE outer products, NG=8 pixel columns block-diagonal per
    matmul: ps[g*16+l, g*16+h] += onehot_lo^T @ onehot_hi over 32 matmuls.
    The weights operand (onehot_lo) must be a single packed free dim ->
    row-major [P, c, l]; the moving operand (onehot_hi) tolerates a strided
    AP -> column-last [P, h, c].
  - one-hot builds are spread over three engines, all sized to ~4.4us/image:
      * hi (column-last): batched DVE is_equal, all-bf16 packed operands ->
        DVE 2x_1p perf mode (0.52 ns/elem).
      * lo columns [0,200): ACT broadcast-expands lo to [P,c,16] bf16 (ACT
        has no packing constraint), then DVE is_equal runs packed at 2x.
      * lo columns [200,224): direct DVE is_equal with a broadcast comparand
        (1x), [224,256): per-column Pool tensor_scalar.
  - the raw [128, 128] PSUM table is copied to SBUF and DMA'd out; the host
    sums the 8 diagonal 16x16 blocks (and the 8 cores — the all-reduce),
    applies the exact f64 cell-averaged Phi map, then the MSE.
"""

import os
import sys

import numpy as np

for _p in ("/opt/trn_rl_repo", "/root/.axon_site/_ro/trn_rl_repo"):
    if os.path.isdir(_p) and _p not in sys.path:
        sys.path.insert(0, _p)

import concourse.bass as bass  # noqa: E402
import concourse.tile as tile  # noqa: E402
from concourse import bacc, mybir  # noqa: E402
from concourse.bass_utils import run_bass_kernel_spmd, axon_active  # noqa: E402

N_CORES = 8
N_IMG = 3
IMG_PIX = 4 * 1 * 256 * 256          # 262144 pixels per image
SHARD = IMG_PIX // N_CORES           # 32768 pixels per core per image
P, T = 128, 256                      # on-chip pixel layout (SHARD = P*T)
W = 16                               # one-hot width (hi and lo)
NG = 8                               # pixel columns per matmul (block-diag)
GRID = W * W                         # 256 fine levels, u = W*hi + lo
SCALE = 255.0                        # u = round(x * 255): exactly the bins
MAGIC = 8388608.0                    # 2**23: float32 round-to-nearest trick
TC = 64                              # columns per one-hot build instruction
A_COLS = 200                         # lo cols via ACT-expand + DVE 2x
D_COLS = 224                         # .. then DVE-direct 1x up to here
ACHUNK = 50                          # expand/build chunk on the ACT path
SIGMA = 0.01
BINS = 256

_CACHE = {}


def _build_program():
    nc = bacc.Bacc(
        "TRN2",
        target_bir_lowering=False,
        debug=not axon_active(),
        num_devices=N_CORES,
    )
    f32 = mybir.dt.float32
    bf16 = mybir.dt.bfloat16
    A = mybir.AluOpType
    CP = mybir.ActivationFunctionType.Copy

    x_d = nc.dram_tensor("x", [N_IMG, P, T], f32, kind="ExternalInput")
    iH_d = nc.dram_tensor("iotaH", [P, W, TC], bf16, kind="ExternalInput")
    iR_d = nc.dram_tensor("iotaR", [P, TC, W], bf16, kind="ExternalInput")
    iRf_d = nc.dram_tensor("iotaRf", [P, TC, W], f32, kind="ExternalInput")
    cnt_d = nc.dram_tensor("cnt", [N_IMG, NG * W, NG * W], f32, kind="ExternalOutput")

    with tile.TileContext(nc) as tc:
        with (
            tc.tile_pool(name="pool", bufs=3) as pool,
            tc.tile_pool(name="cpool", bufs=1) as cpool,
            tc.tile_pool(name="psum", bufs=2, space=bass.MemorySpace.PSUM) as pp,
        ):
            iotaH = cpool.tile([P, W, TC], bf16, tag="iotaH")
            nc.sync.dma_start(iotaH[:], iH_d[:])
            iotaR = cpool.tile([P, TC, W], bf16, tag="iotaR")
            nc.sync.dma_start(iotaR[:], iR_d[:])
            iotaRf = cpool.tile([P, TC, W], f32, tag="iotaRf")
            nc.sync.dma_start(iotaRf[:], iRf_d[:])

            for i in range(N_IMG):
                x = pool.tile([P, T], f32, tag="x")
                nc.sync.dma_start(x[:], x_d[i])

                # u = round(x*255) and hi = round((u-7.5)/16) via magic-adds
                t0 = pool.tile([P, T], f32, tag="t0")
                nc.scalar.activation(t0[:], x[:], CP, bias=MAGIC, scale=SCALE)
                u = pool.tile([P, T], f32, tag="u")
                nc.scalar.activation(u[:], t0[:], CP, bias=-MAGIC)
                t1 = pool.tile([P, T], f32, tag="t1")
                nc.scalar.activation(
                    t1[:], u[:], CP, bias=8.0 - (W / 2.0 - 0.5) / W, scale=1.0 / W
                )
                t2 = pool.tile([P, T], f32, tag="t2")
                nc.scalar.activation(t2[:], t1[:], CP, bias=MAGIC)
                hi = pool.tile([P, T], bf16, tag="hi")
                nc.scalar.activation(hi[:], t2[:], CP, bias=-(MAGIC + 8.0))
                lo = pool.tile([P, T], f32, tag="lo")
                nc.vector.scalar_tensor_tensor(
                    lo[:], hi[:], -float(W), u[:], A.mult, A.add
                )

                # hi one-hot, column-last [P, w, c]: all-bf16 packed -> DVE 2x
                Ohi = pool.tile([P, W, T], bf16, tag="Ohi")
                for c0 in range(0, T, TC):
                    nc.vector.tensor_tensor(
                        Ohi[:, :, c0 : c0 + TC],
                        iotaH[:],
                        hi[:, None, c0 : c0 + TC].broadcast_to([P, W, TC]),
                        A.is_equal,
                    )

                # lo one-hot, row-major [P, c, w]
                Olo = pool.tile([P, T, W], bf16, tag="Olo")
                lo_rep = pool.tile([P, A_COLS, W], bf16, tag="lo_rep")
                for c0 in range(0, A_COLS, ACHUNK):
                    n = min(ACHUNK, A_COLS - c0)
                    nc.scalar.activation(
                        lo_rep[:, c0 : c0 + n, :],
                        lo[:, c0 : c0 + n, None].broadcast_to([P, n, W]),
                        CP,
                        bias=0.0,
                    )
                    nc.vector.tensor_tensor(
                        Olo[:, c0 : c0 + n, :],
                        iotaR[:, 0:n, :],
                        lo_rep[:, c0 : c0 + n, :],
                        A.is_equal,
                    )
                nc.vector.tensor_tensor(
                    Olo[:, A_COLS:D_COLS, :],
                    iotaRf[:, 0 : D_COLS - A_COLS, :],
                    lo[:, A_COLS:D_COLS, None].broadcast_to(
                        [P, D_COLS - A_COLS, W]
                    ),
                    A.is_equal,
                )
                for c in range(D_COLS, T):
                    nc.gpsimd.tensor_scalar(
                        Olo[:, c, :], iotaRf[:, 0, :], lo[:, c : c + 1],
                        None, A.is_equal,
                    )

                ps = pp.tile([NG * W, NG * W], f32, tag="ps")
                nmm = T // NG
                for m in range(nmm):
                    lhsT = Olo[:, m * NG : (m + 1) * NG, :]
                    rhs = Ohi[:, :, m * NG : (m + 1) * NG].rearrange(
                        "p w c -> p c w"
                    )
                    nc.tensor.matmul(
                        ps[:], lhsT, rhs, start=(m == 0), stop=(m == nmm - 1)
                    )
                res = pool.tile([NG * W, NG * W], f32, tag="res")
                nc.scalar.activation(res[:], ps[:], CP, bias=0.0)
                nc.sync.dma_start(cnt_d[i], res[:])

    nc.compile()
    return nc


def _phi():
    """f64 [GRID, BINS] map: cell-averaged smooth-histogram contribution."""
    b = np.arange(BINS, dtype=np.float64)
    step = SCALE / 255.0
    u_grid = np.arange(GRID, dtype=np.float64)
    nsub = 17
    offs = np.linspace(-0.5, 0.5, nsub)
    wts = np.ones(nsub)
    wts[1:-1:2], wts[2:-1:2] = 4.0, 2.0
    wts /= wts.sum()
    phi = np.zeros((GRID, BINS))
    for o, ws in zip(offs, wts):
        diff = ((u_grid + o)[:, None] - step * b[None, :]) / SCALE
        w = np.exp(-0.5 * (diff / SIGMA) ** 2)
        phi += ws * (w / (w.sum(axis=1, keepdims=True) + 1e-8))
    return phi


def _iotas_np():
    npbf16 = mybir.dt.np(mybir.dt.bfloat16)
    w = np.arange(W, dtype=np.float32)
    iH = np.broadcast_to(w[None, :, None], (P, W, TC)).astype(npbf16)
    iR = np.broadcast_to(w[None, None, :], (P, TC, W)).astype(npbf16)
    iRf = np.broadcast_to(w[None, None, :], (P, TC, W)).astype(np.float32)
    return (
        np.ascontiguousarray(iH),
        np.ascontiguousarray(iR),
        np.ascontiguousarray(iRf),
    )


def _get_state():
    if "nc" not in _CACHE:
        _CACHE["nc"] = _build_program()
        _CACHE["phi"] = _phi()
        _CACHE["iota"] = _iotas_np()
    return _CACHE["nc"], _CACHE["phi"], _CACHE["iota"]


def _run_device(images, trace=False):
    """images: [3, IMG_PIX] f32 -> (results, counts [3, GRID] f64)."""
    nc, phi, (iH, iR, iRf) = _get_state()
    in_maps = []
    for k in range(N_CORES):
        shard = images[:, k * SHARD : (k + 1) * SHARD].reshape(N_IMG, P, T)
        in_maps.append(
            {
                "x": np.ascontiguousarray(shard),
                "iotaH": iH,
                "iotaR": iR,
                "iotaRf": iRf,
            }
        )
    res = run_bass_kernel_spmd(nc, in_maps, list(range(N_CORES)), trace=trace)
    cnt = np.zeros((N_IMG, GRID), dtype=np.float64)
    for k in range(N_CORES):
        ps = res.results[k]["cnt"].astype(np.float64)  # [3, 128(l), 128(h)]
        for g in range(NG):
            blk = ps[:, g * W : (g + 1) * W, g * W : (g + 1) * W]
            # ps[l, h] -> cnt[u = 16*h + l]
            cnt += blk.transpose(0, 2, 1).reshape(N_IMG, GRID)
    return res, cnt


def kernel(fused_image, ir_image, visible_gray):
    imgs = np.stack(
        [
            np.asarray(fused_image, dtype=np.float32).reshape(-1),
            np.asarray(ir_image, dtype=np.float32).reshape(-1),
            np.asarray(visible_gray, dtype=np.float32).reshape(-1),
        ]
    )
    _, cnt = _run_device(imgs)
    _, phi, _ = _get_state()
    hists = cnt @ phi  # [3, 256] f64
    hf, hi_, hv = hists
    loss_ir = np.mean((hf - hi_) ** 2)
    loss_vis = np.mean((hf - hv) ** 2)
    return np.array(0.5 * loss_ir + 0.5 * loss_vis, dtype=np.float32)


# revision 11
# speedup vs baseline: 1.0372x; 1.0372x over previous
"""Trainium2 Bass kernel for nn_ContrastLoss (smooth-histogram contrast loss).

Algorithm
---------
reference computes, per image:  hist[b] = sum_p w(x_p,b) / (S_p + 1e-8),
w = exp(-0.5*((x - c_b)/sigma)^2), c_b = b/255, sigma = 0.01, S_p = sum_b w,
followed by MSEs between the three histograms.

hist is a fixed linear map of the count histogram of u = round(x * 255)
in [0, 255] (256 levels = the bin centers themselves; quantization error on
the loss is ~5e-4 rel, far inside tolerance):
    hist[b] = sum_u cnt[u] * Phi[u, b]
The device only needs cnt[256] per image — a pure counting problem.

Device kernel (SPMD over 8 cores, data-parallel over pixels):
  - per core/image, 32768 pixels in SBUF [128, 256]; u = round(255 x) via the
    2^23 magic-add on ACT; split u = 16*hi + lo.  lo is kept MAGIC-shifted
    (lo_m = u + 2^23 - 16*hi) so it needs no extra ACT pass: the expand
    subtracts 2^23 via its bias, the Pool compare uses a shifted iota.
  - counting via PE outer products, NG=8 pixel columns block-diagonal per
    matmul: ps[g*16+l, g*16+h] += onehot_lo^T @ onehot_hi over 32 matmuls.
    The weights operand (onehot_lo) must be a single packed free dim ->
    row-major [P, c, l]; the moving operand (onehot_hi) tolerates a strided
    AP -> column-last [P, h, c].
  - one-hot builds are spread over three engines, each ~4.7us/image:
      * hi (column-last): batched DVE is_equal, all-bf16 packed operands ->
        DVE 2x_1p perf mode (0.52 ns/elem).
      * lo columns [0,220): ACT broadcast-expands lo to [P,c,16] bf16 (ACT
        has no packing constraint), then DVE is_equal runs packed at 2x.
      * lo columns [220,256): per-column Pool tensor_scalar.
  - emission order is software-pipelined (all input DMAs first, prep of
    image i+1 ahead of builds of image i) so the three engine streams and
    the SP DMA queue never block each other across images.
  - the raw [128, 128] PSUM table is copied to SBUF (Pool) and DMA'd out;
    the host sums the 8 diagonal 16x16 blocks (and the 8 cores — the
    all-reduce), applies the exact f64 cell-averaged Phi map, then the MSE.
"""

import os
import sys

import numpy as np

for _p in ("/opt/trn_rl_repo", "/root/.axon_site/_ro/trn_rl_repo"):
    if os.path.isdir(_p) and _p not in sys.path:
        sys.path.insert(0, _p)

import concourse.bass as bass  # noqa: E402
import concourse.tile as tile  # noqa: E402
from concourse import bacc, mybir  # noqa: E402
from concourse.bass_utils import run_bass_kernel_spmd, axon_active  # noqa: E402

N_CORES = 8
N_IMG = 3
IMG_PIX = 4 * 1 * 256 * 256          # 262144 pixels per image
SHARD = IMG_PIX // N_CORES           # 32768 pixels per core per image
P, T = 128, 256                      # on-chip pixel layout (SHARD = P*T)
W = 16                               # one-hot width (hi and lo)
NG = 8                               # pixel columns per matmul (block-diag)
GRID = W * W                         # 256 fine levels, u = W*hi + lo
SCALE = 255.0                        # u = round(x * 255): exactly the bins
MAGIC = 8388608.0                    # 2**23: float32 round-to-nearest trick
TC = 64                              # columns per hi one-hot instruction
A_COLS = 220                         # lo cols via ACT-expand + DVE 2x
ACHUNK = 55                          # expand/build chunk on the ACT path
SIGMA = 0.01
BINS = 256

_CACHE = {}


def _build_program():
    nc = bacc.Bacc(
        "TRN2",
        target_bir_lowering=False,
        debug=not axon_active(),
        num_devices=N_CORES,
    )
    f32 = mybir.dt.float32
    bf16 = mybir.dt.bfloat16
    A = mybir.AluOpType
    CP = mybir.ActivationFunctionType.Copy

    x_d = nc.dram_tensor("x", [N_IMG, P, T], f32, kind="ExternalInput")
    iH_d = nc.dram_tensor("iotaH", [P, W, TC], bf16, kind="ExternalInput")
    iR_d = nc.dram_tensor("iotaR", [P, ACHUNK, W], bf16, kind="ExternalInput")
    iRs_d = nc.dram_tensor("iotaRs", [P, W], f32, kind="ExternalInput")
    cnt_d = nc.dram_tensor("cnt", [N_IMG, NG * W, NG * W], f32, kind="ExternalOutput")

    with tile.TileContext(nc) as tc:
        with (
            tc.tile_pool(name="pool", bufs=3) as pool,
            tc.tile_pool(name="cpool", bufs=1) as cpool,
            tc.tile_pool(name="psum", bufs=2, space=bass.MemorySpace.PSUM) as pp,
        ):
            iotaH = cpool.tile([P, W, TC], bf16, tag="iotaH")
            nc.sync.dma_start(iotaH[:], iH_d[:])
            iotaR = cpool.tile([P, ACHUNK, W], bf16, tag="iotaR")
            nc.sync.dma_start(iotaR[:], iR_d[:])
            iotaRs = cpool.tile([P, W], f32, tag="iotaRs")
            nc.sync.dma_start(iotaRs[:], iRs_d[:])

            xs, t0s, his, lms = {}, {}, {}, {}
            for i in range(N_IMG):
                xs[i] = pool.tile([P, T], f32, tag="x", name=f"x{i}")
                nc.sync.dma_start(xs[i][:], x_d[i])

            def stage_a(i):
                # u = round(x*255) and hi = round((u-7.5)/16) via magic-adds;
                # lo = u - 16*hi in one DVE stt
                t0 = pool.tile([P, T], f32, tag="t0")
                nc.scalar.activation(t0[:], xs[i][:], CP, bias=MAGIC, scale=SCALE)
                t0s[i] = pool.tile([P, T], f32, tag="u", name=f"u{i}")
                nc.scalar.activation(t0s[i][:], t0[:], CP, bias=-MAGIC)
                t1 = pool.tile([P, T], f32, tag="t1")
                nc.scalar.activation(
                    t1[:],
                    t0s[i][:],
                    CP,
                    bias=8.0 - (W / 2.0 - 0.5) / W,
                    scale=1.0 / W,
                )
                t2 = pool.tile([P, T], f32, tag="t2")
                nc.scalar.activation(t2[:], t1[:], CP, bias=MAGIC)
                his[i] = pool.tile([P, T], bf16, tag="hi", name=f"hi{i}")
                nc.scalar.activation(his[i][:], t2[:], CP, bias=-(MAGIC + 8.0))
                lms[i] = pool.tile([P, T], f32, tag="lm", name=f"lm{i}")
                nc.vector.scalar_tensor_tensor(
                    lms[i][:], his[i][:], -float(W), t0s[i][:], A.mult, A.add
                )

            def stage_bc(i):
                hi, lm = his[i], lms[i]
                # hi one-hot, column-last [P, w, c]: all-bf16 packed -> DVE 2x
                Ohi = pool.tile([P, W, T], bf16, tag="Ohi")
                for c0 in range(0, T, TC):
                    nc.vector.tensor_tensor(
                        Ohi[:, :, c0 : c0 + TC],
                        iotaH[:],
                        hi[:, None, c0 : c0 + TC].broadcast_to([P, W, TC]),
                        A.is_equal,
                    )
                # lo one-hot, row-major [P, c, w]
                Olo = pool.tile([P, T, W], bf16, tag="Olo")
                lo_rep = pool.tile([P, A_COLS, W], bf16, tag="lo_rep")
                for c0 in range(0, A_COLS, ACHUNK):
                    n = min(ACHUNK, A_COLS - c0)
                    nc.scalar.activation(
                        lo_rep[:, c0 : c0 + n, :],
                        lm[:, c0 : c0 + n, None].broadcast_to([P, n, W]),
                        CP,
                        bias=0.0,
                    )
                    nc.vector.tensor_tensor(
                        Olo[:, c0 : c0 + n, :],
                        iotaR[:, 0:n, :],
                        lo_rep[:, c0 : c0 + n, :],
                        A.is_equal,
                    )
                for c in range(A_COLS, T):
                    nc.gpsimd.tensor_scalar(
                        Olo[:, c, :], iotaRs[:], lm[:, c : c + 1],
                        None, A.is_equal,
                    )

                ps = pp.tile([NG * W, NG * W], f32, tag="ps")
                nmm = T // NG
                for m in range(nmm):
                    lhsT = Olo[:, m * NG : (m + 1) * NG, :]
                    rhs = Ohi[:, :, m * NG : (m + 1) * NG].rearrange(
                        "p w c -> p c w"
                    )
                    nc.tensor.matmul(
                        ps[:], lhsT, rhs, start=(m == 0), stop=(m == nmm - 1)
                    )
                res = pool.tile([NG * W, NG * W], f32, tag="res")
                nc.scalar.activation(res[:], ps[:], CP, bias=0.0)
                nc.sync.dma_start(cnt_d[i], res[:])

            # software pipeline: prep of image i+1 ahead of builds of image i
            stage_a(0)
            stage_a(1)
            stage_bc(0)
            stage_a(2)
            stage_bc(1)
            stage_bc(2)

    nc.compile()
    return nc


def _phi():
    """f64 [GRID, BINS] map: cell-averaged smooth-histogram contribution."""
    b = np.arange(BINS, dtype=np.float64)
    step = SCALE / 255.0
    u_grid = np.arange(GRID, dtype=np.float64)
    nsub = 17
    offs = np.linspace(-0.5, 0.5, nsub)
    wts = np.ones(nsub)
    wts[1:-1:2], wts[2:-1:2] = 4.0, 2.0
    wts /= wts.sum()
    phi = np.zeros((GRID, BINS))
    for o, ws in zip(offs, wts):
        diff = ((u_grid + o)[:, None] - step * b[None, :]) / SCALE
        w = np.exp(-0.5 * (diff / SIGMA) ** 2)
        phi += ws * (w / (w.sum(axis=1, keepdims=True) + 1e-8))
    return phi


def _iotas_np():
    npbf16 = mybir.dt.np(mybir.dt.bfloat16)
    w = np.arange(W, dtype=np.float32)
    iH = np.broadcast_to(w[None, :, None], (P, W, TC)).astype(npbf16)
    iR = np.broadcast_to(w[None, None, :], (P, ACHUNK, W)).astype(npbf16)
    iRs = np.broadcast_to(w[None, :], (P, W)).astype(np.float32)
    return (
        np.ascontiguousarray(iH),
        np.ascontiguousarray(iR),
        np.ascontiguousarray(iRs),
    )


def _get_state():
    if "nc" not in _CACHE:
        _CACHE["nc"] = _build_program()
        _CACHE["phi"] = _phi()
        _CACHE["iota"] = _iotas_np()
    return _CACHE["nc"], _CACHE["phi"], _CACHE["iota"]


def _run_device(images, trace=False):
    """images: [3, IMG_PIX] f32 -> (results, counts [3, GRID] f64)."""
    nc, phi, (iH, iR, iRs) = _get_state()
    in_maps = []
    for k in range(N_CORES):
        shard = images[:, k * SHARD : (k + 1) * SHARD].reshape(N_IMG, P, T)
        in_maps.append(
            {
                "x": np.ascontiguousarray(shard),
                "iotaH": iH,
                "iotaR": iR,
                "iotaRs": iRs,
            }
        )
    res = run_bass_kernel_spmd(nc, in_maps, list(range(N_CORES)), trace=trace)
    cnt = np.zeros((N_IMG, GRID), dtype=np.float64)
    for k in range(N_CORES):
        ps = res.results[k]["cnt"].astype(np.float64)  # [3, 128(l), 128(h)]
        for g in range(NG):
            blk = ps[:, g * W : (g + 1) * W, g * W : (g + 1) * W]
            # ps[l, h] -> cnt[u = 16*h + l]
            cnt += blk.transpose(0, 2, 1).reshape(N_IMG, GRID)
    return res, cnt


def kernel(fused_image, ir_image, visible_gray):
    imgs = np.stack(
        [
            np.asarray(fused_image, dtype=np.float32).reshape(-1),
            np.asarray(ir_image, dtype=np.float32).reshape(-1),
            np.asarray(visible_gray, dtype=np.float32).reshape(-1),
        ]
    )
    _, cnt = _run_device(imgs)
    _, phi, _ = _get_state()
    hists = cnt @ phi  # [3, 256] f64
    hf, hi_, hv = hists
    loss_ir = np.mean((hf - hi_) ** 2)
    loss_vis = np.mean((hf - hv) ** 2)
    return np.array(0.5 * loss_ir + 0.5 * loss_vis, dtype=np.float32)


# revision 14
# speedup vs baseline: 1.1234x; 1.0831x over previous
"""Trainium2 Bass kernel for nn_ContrastLoss (smooth-histogram contrast loss).

Algorithm
---------
reference computes, per image:  hist[b] = sum_p w(x_p,b) / (S_p + 1e-8),
w = exp(-0.5*((x - c_b)/sigma)^2), c_b = b/255, sigma = 0.01, S_p = sum_b w,
followed by MSEs between the three histograms.

hist is a fixed linear map of the count histogram of u = round(x * 255)
in [0, 255] (256 levels = the bin centers themselves; quantization error on
the loss is ~5e-4 rel, far inside tolerance):
    hist[b] = sum_u cnt[u] * Phi[u, b]
The device only needs cnt[256] per image — a pure counting problem.

Device kernel (SPMD over 8 cores, data-parallel over pixels):
  - per core/image, 32768 pixels in SBUF [128, 256]; u = round(255 x) via the
    2^23 magic-add on ACT; split u = 16*hi + lo (hi via a second magic-add on
    ACT, lo via one DVE scalar_tensor_tensor, both exact small ints in bf16).
  - counting via PE outer products, NG=8 pixel columns block-diagonal per
    matmul m: ps += onehot_lo(group m)^T @ onehot_hi(cols of m).
    Weights APs must collapse to ONE packed free dim, so onehot_lo lives as
    Olo[p, m, l, g] (l-major inside each 8-column group): [8,16]x[1,8]
    collapses to a 128-long stride-1 run.  The moving operand tolerates a
    strided AP, so onehot_hi lives column-last as Ohi[p, w, c].  The PSUM
    table comes out index-permuted (ps[8l+g, 16g+h]) — host unscrambles.
  - BOTH one-hot layouts give batched DVE is_equal instructions whose
    operands are all 2-byte, SBUF, innermost-stride-1 -> DVE 2x_1p perf mode
    (0.52 ns/elem).  Pool builds the last 40 hi columns via per-column
    tensor_scalar (f32 comparand) to offload DVE; ACT only does prep + the
    PSUM->SBUF copy.
  - DMAs: one tiny f32 iota seed (issued on the DVE queue, which then
    derives the bf16 iota tiles on-device during its idle head), x image 0
    alone (critical path) then images 1+2 in one DMACopy — each DMACopy
    costs ~625ns on the shared HWDGE device, so fewer + smaller is faster.
  - host sums the 8 diagonal blocks of the permuted table (and the 8 cores —
    the all-reduce), applies the exact f64 cell-averaged Phi map, then MSE.
"""

import os
import sys

import numpy as np

for _p in ("/opt/trn_rl_repo", "/root/.axon_site/_ro/trn_rl_repo"):
    if os.path.isdir(_p) and _p not in sys.path:
        sys.path.insert(0, _p)

import concourse.bass as bass  # noqa: E402
import concourse.tile as tile  # noqa: E402
from concourse import bacc, mybir  # noqa: E402
from concourse.bass_utils import run_bass_kernel_spmd, axon_active  # noqa: E402

N_CORES = 8
N_IMG = 3
IMG_PIX = 4 * 1 * 256 * 256          # 262144 pixels per image
SHARD = IMG_PIX // N_CORES           # 32768 pixels per core per image
P, T = 128, 256                      # on-chip pixel layout (SHARD = P*T)
W = 16                               # one-hot width (hi and lo)
NG = 8                               # pixel columns per matmul (block-diag)
NGRP = T // NG                       # 32 column groups per image
GRID = W * W                         # 256 fine levels, u = W*hi + lo
SCALE = 255.0                        # u = round(x * 255): exactly the bins
MAGIC = 8388608.0                    # 2**23: float32 round-to-nearest trick
TC = 64                              # hi columns per DVE build instruction
G_COLS = 40                          # trailing hi columns built on Pool
MCHUNK = 16                          # lo groups per DVE build instruction
SIGMA = 0.01
BINS = 256

_CACHE = {}


def _build_program():
    nc = bacc.Bacc(
        "TRN2",
        target_bir_lowering=False,
        debug=not axon_active(),
        num_devices=N_CORES,
    )
    f32 = mybir.dt.float32
    bf16 = mybir.dt.bfloat16
    A = mybir.AluOpType
    CP = mybir.ActivationFunctionType.Copy

    x_d = nc.dram_tensor("x", [N_IMG, P, T], f32, kind="ExternalInput")
    seed_d = nc.dram_tensor("seed", [P, W], f32, kind="ExternalInput")
    cnt_d = nc.dram_tensor("cnt", [N_IMG, NG * W, NG * W], f32, kind="ExternalOutput")

    with tile.TileContext(nc) as tc:
        with (
            tc.tile_pool(name="pool", bufs=3) as pool,
            tc.tile_pool(name="cpool", bufs=1) as cpool,
            tc.tile_pool(name="psum", bufs=2, space=bass.MemorySpace.PSUM) as pp,
        ):
            # tiny seed DMA on the DVE queue; DVE then derives the iota
            # tiles during its otherwise-idle head (before image 0 prep
            # lands).  iotaRs (f32, for Pool's comparisons) IS the seed.
            iotaRs = cpool.tile([P, W], f32, tag="iotaRs")
            nc.scalar.dma_start(iotaRs[:], seed_d[:])
            iotaWb = cpool.tile([P, W], bf16, tag="iotaWb")
            nc.vector.tensor_scalar(iotaWb[:], iotaRs[:], 1.0, None, A.mult)
            iotaL2 = cpool.tile([P, W, NG], bf16, tag="iotaL2")
            nc.vector.tensor_scalar(
                iotaL2[:],
                iotaWb[:, :, None].broadcast_to([P, W, NG]),
                1.0, None, A.mult,
            )
            iotaH = cpool.tile([P, W, TC], bf16, tag="iotaH")
            nc.vector.tensor_scalar(
                iotaH[:],
                iotaWb[:, :, None].broadcast_to([P, W, TC]),
                1.0, None, A.mult,
            )

            xs, hfs, hbs, lbs = {}, {}, {}, {}
            x12 = cpool.tile([P, 2, T], f32, tag="x12")
            xs[0] = pool.tile([P, T], f32, tag="x0", name="x0")
            nc.sync.dma_start(xs[0][:], x_d[0])
            nc.sync.dma_start(x12[:], x_d[1:3].rearrange("i p t -> p i t"))
            xs[1] = x12[:, 0, :]
            xs[2] = x12[:, 1, :]

            def stage_a(i):
                # u = round(x*255), hi = round((u-7.5)/16) (magic-adds, ACT);
                # lo = u - 16*hi (DVE stt).  hi kept in f32 (Pool comparand)
                # and bf16 (DVE comparand); lo in bf16.
                t0 = pool.tile([P, T], f32, tag="t0")
                nc.scalar.activation(t0[:], xs[i][:], CP, bias=MAGIC, scale=SCALE)
                u = pool.tile([P, T], f32, tag="u")
                nc.scalar.activation(u[:], t0[:], CP, bias=-MAGIC)
                t1 = pool.tile([P, T], f32, tag="t1")
                nc.scalar.activation(
                    t1[:], u[:], CP, bias=8.0 - (W / 2.0 - 0.5) / W, scale=1.0 / W
                )
                t2 = pool.tile([P, T], f32, tag="t2")
                nc.scalar.activation(t2[:], t1[:], CP, bias=MAGIC)
                hfs[i] = pool.tile([P, T], f32, tag="hf", name=f"hf{i}")
                nc.scalar.activation(hfs[i][:], t2[:], CP, bias=-(MAGIC + 8.0))
                hbs[i] = pool.tile([P, T], bf16, tag="hb", name=f"hb{i}")
                nc.scalar.activation(hbs[i][:], hfs[i][:], CP, bias=0.0)
                lbs[i] = pool.tile([P, T], bf16, tag="lb", name=f"lb{i}")
                nc.vector.scalar_tensor_tensor(
                    lbs[i][:], hfs[i][:], -float(W), u[:], A.mult, A.add
                )

            def stage_bc(i):
                hf, hb, lb = hfs[i], hbs[i], lbs[i]
                # lo one-hot in weights layout [P, m, l, g]: all-bf16 packed
                Olo = pool.tile([P, NGRP, W, NG], bf16, tag="Olo")
                lbg = lb.rearrange("p (m g) -> p m g", g=NG)
                for m0 in range(0, NGRP, MCHUNK):
                    nc.vector.tensor_tensor(
                        Olo[:, m0 : m0 + MCHUNK, :, :],
                        iotaL2[:, None, :, :].broadcast_to([P, MCHUNK, W, NG]),
                        lbg[:, m0 : m0 + MCHUNK, None, :].broadcast_to(
                            [P, MCHUNK, W, NG]
                        ),
                        A.is_equal,
                    )
                # hi one-hot, column-last [P, w, c]: DVE 2x + Pool tail
                Ohi = pool.tile([P, W, T], bf16, tag="Ohi")
                dve_hi = T - G_COLS
                for c0 in range(0, dve_hi, TC):
                    n = min(TC, dve_hi - c0)
                    nc.vector.tensor_tensor(
                        Ohi[:, :, c0 : c0 + n],
                        iotaH[:, :, 0:n],
                        hb[:, None, c0 : c0 + n].broadcast_to([P, W, n]),
                        A.is_equal,
                    )
                for c in range(dve_hi, T):
                    nc.gpsimd.tensor_scalar(
                        Ohi[:, :, c], iotaRs[:], hf[:, c : c + 1],
                        None, A.is_equal,
                    )

                ps = pp.tile([NG * W, NG * W], f32, tag="ps")
                for m in range(NGRP):
                    lhsT = Olo[:, m, :, :]
                    rhs = Ohi[:, :, m * NG : (m + 1) * NG].rearrange(
                        "p w c -> p c w"
                    )
                    nc.tensor.matmul(
                        ps[:], lhsT, rhs, start=(m == 0), stop=(m == NGRP - 1)
                    )
                res = pool.tile([NG * W, NG * W], f32, tag="res")
                nc.scalar.activation(res[:], ps[:], CP, bias=0.0)
                nc.sync.dma_start(cnt_d[i], res[:])

            # software pipeline: prep of image i+1 ahead of builds of image i
            stage_a(0)
            stage_a(1)
            stage_bc(0)
            stage_a(2)
            stage_bc(1)
            stage_bc(2)

    nc.compile()
    return nc


def _phi():
    """f64 [GRID, BINS] map: cell-averaged smooth-histogram contribution."""
    b = np.arange(BINS, dtype=np.float64)
    step = SCALE / 255.0
    u_grid = np.arange(GRID, dtype=np.float64)
    nsub = 17
    offs = np.linspace(-0.5, 0.5, nsub)
    wts = np.ones(nsub)
    wts[1:-1:2], wts[2:-1:2] = 4.0, 2.0
    wts /= wts.sum()
    phi = np.zeros((GRID, BINS))
    for o, ws in zip(offs, wts):
        diff = ((u_grid + o)[:, None] - step * b[None, :]) / SCALE
        w = np.exp(-0.5 * (diff / SIGMA) ** 2)
        phi += ws * (w / (w.sum(axis=1, keepdims=True) + 1e-8))
    return phi


def _seed_np():
    return np.ascontiguousarray(
        np.broadcast_to(np.arange(W, dtype=np.float32)[None, :], (P, W))
    )


def _get_state():
    if "nc" not in _CACHE:
        _CACHE["nc"] = _build_program()
        _CACHE["phi"] = _phi()
        _CACHE["seed"] = _seed_np()
    return _CACHE["nc"], _CACHE["phi"], _CACHE["seed"]


def _run_device(images, trace=False):
    """images: [3, IMG_PIX] f32 -> (results, counts [3, GRID] f64)."""
    nc, phi, seed = _get_state()
    in_maps = []
    for k in range(N_CORES):
        shard = images[:, k * SHARD : (k + 1) * SHARD].reshape(N_IMG, P, T)
        in_maps.append({"x": np.ascontiguousarray(shard), "seed": seed})
    res = run_bass_kernel_spmd(nc, in_maps, list(range(N_CORES)), trace=trace)
    cnt = np.zeros((N_IMG, GRID), dtype=np.float64)
    for k in range(N_CORES):
        ps = res.results[k]["cnt"].astype(np.float64)  # [3, 128, 128]
        # ps[8l+g, 16g+h] -> cnt[u = 16h+l]
        psr = ps.reshape(N_IMG, W, NG, NG, W)  # [i, l, g, g', h]
        for g in range(NG):
            cnt += psr[:, :, g, g, :].transpose(0, 2, 1).reshape(N_IMG, GRID)
    return res, cnt


def kernel(fused_image, ir_image, visible_gray):
    imgs = np.stack(
        [
            np.asarray(fused_image, dtype=np.float32).reshape(-1),
            np.asarray(ir_image, dtype=np.float32).reshape(-1),
            np.asarray(visible_gray, dtype=np.float32).reshape(-1),
        ]
    )
    _, cnt = _run_device(imgs)
    _, phi, _ = _get_state()
    hists = cnt @ phi  # [3, 256] f64
    hf, hi_, hv = hists
    loss_ir = np.mean((hf - hi_) ** 2)
    loss_vis = np.mean((hf - hv) ** 2)
    return np.array(0.5 * loss_ir + 0.5 * loss_vis, dtype=np.float32)
